# revision 3
# baseline (speedup 1.0000x reference)
"""MergeAdapter (moe_routing) Trainium2 Bass kernel — transposed-output fp8 design.

Reference computation (per instance n):
    wd = sum_k prob[n,k] * w_down[k]   (D, H)     bd = sum_k prob[n,k] * b_down[k]
    wu = sum_k prob[n,k] * w_up[k]     (H, D)     bu = sum_k prob[n,k] * b_up[k]
    out[n] = x[n] + relu(x[n] @ wd.T + bd) @ wu.T + bu

Sharding: data-parallel over N=16 -> 2 instances/core on 8 cores, full expert
banks everywhere, no communication.

Design (vs the 141us baseline this replaces):
  - ALL device compute happens in "transposed" space out'[h, s] instead of
    out[s, h].  Then:
      * the skip-add operand is exactly the already-loaded xT tile -> the
        8 MiB second load of x (natural layout) disappears, as do the PE
        identity-matmul skip pass and the ones-row bias pass (-64K PE cycles)
      * b_up becomes a per-partition scalar -> rides the epilogue for free
      * the store is contiguous in the transposed layout; the host undoes the
        transpose (pure data movement, same as the host-side input transposes)
  - weights travel as fp8e4m3 (4 MiB instead of 8): the residual path
    tolerates fp8 easily (gate is 2e-2; residual is ~3% of |out|)
  - expert-bank merging moves from 47us of DVE chains to ~7us of PE
    scaled-identity DoubleRow matmuls: merged[m,f] = sum_i p8[2kk+i] *
    bank_pair[m,i,f] accumulated over kk in PSUM, drained fp8 by DVE
  - mm2 contraction (D=256) is a single fp8 DoubleRow matmul per tile at
    0.5 cycles/row; mm1 keeps fp16 x (mixed fp8 lhsT x fp16 rhs at 1.0)
  - epilogue (psum + b_up + xT skip -> fp16 out) is one fused
    scalar_tensor_tensor on DVE for part of the tiles, ACT(bias copy)+Pool(add)
    for the rest, so no single engine owns the full 32K-elem/partition pass
  - HBM traffic: 8 (xT fp16) + 4 (w fp8) + 8 (out fp16) = 20 MiB/core vs 32.
Engine budget @ steady state: DMA ~60-72us (bound), PE ~41us, DVE ~35us,
ACT ~25us, Pool ~30us.
"""
import os
import sys

for _p in ("/opt/trn_rl_repo",):
    if os.path.isdir(_p) and _p not in sys.path:
        sys.path.insert(0, _p)

import ml_dtypes
import numpy as np

import concourse.mybir as mybir
import concourse.tile as tile
from concourse import bacc
from concourse.bass_utils import run_bass_kernel_spmd

N, S, H, K, D = 16, 2048, 1024, 8, 256
NCORES = 8
NPC = N // NCORES          # instances per core
IC = H // 128              # h-chunks (contraction chunks of mm1; partition tiles of out')
OC = D // 128              # d-chunks (partition tiles of relu1; contraction of mm2)
SCW = 512                  # free-dim tile width (psum bank)
SC = S // SCW              # s-chunks
KK = K // 2                # expert pairs (DoubleRow merges 2 experts/pass)
MCW = 512                  # merge psum chunk width over the flattened bank free dim

F32 = mybir.dt.float32
F16 = mybir.dt.float16
F8 = mybir.dt.float8e4
np16 = np.float16
np8 = ml_dtypes.float8_e4m3

_CACHE: dict = {}
# epilogue routing: fraction of (sc,hc) tiles on the fused DVE path; the rest
# go ACT(bias)+Pool(add).  ablate: None | "dma_only" | "compute_only"
OPTS = {"dve_frac": 0.5, "ablate": None}


def _emit(nc, tc, tens, loop_t=None):
    (xT_d, wd8_d, wu8_d, pid_d, bd_d, bu_d, pkn_d, out_d) = tens
    DR = mybir.MatmulPerfMode.DoubleRow
    with (
        tc.tile_pool(name="consts", bufs=1) as consts,
        tc.tile_pool(name="banks", bufs=2 * KK) as banks,
        tc.tile_pool(name="work", bufs=1) as work,
        tc.tile_pool(name="xtp", bufs=1) as xtp,
        tc.tile_pool(name="obp", bufs=6) as obp,
        tc.tile_pool(name="psm", bufs=2, space="PSUM") as psm,
        tc.tile_pool(name="ps1", bufs=2, space="PSUM") as ps1p,
        tc.tile_pool(name="ps2", bufs=3, space="PSUM") as ps2p,
        tc.tile_pool(name="pst", bufs=1, space="PSUM") as pstiny,
    ):
        bd_t = consts.tile([K, D], F32, tag="bd")
        bu_t = consts.tile([K, H], F32, tag="bu")
        pkn_t = consts.tile([K, NPC], F32, tag="pkn")
        pid_t = consts.tile([128, NPC, KK, 2, 128], F8, tag="pid")
        nc.sync.dma_start(bd_t[:], bd_d.ap())
        nc.sync.dma_start(bu_t[:], bu_d.ap())
        nc.sync.dma_start(pkn_t[:], pkn_d.ap())
        nc.sync.dma_start(pid_t[:], pid_d.ap())

        if loop_t is not None:
            loop_cm = tc.For_i(0, loop_t, 1, hint_engines=tuple(
                getattr(mybir.EngineType, e)
                for e in ("PE", "DVE", "Activation", "SP", "Pool")))
        else:
            import contextlib
            loop_cm = contextlib.nullcontext()

        ABL = OPTS["ablate"]
        with loop_cm:
            if ABL == "dma_only":
                # loads + equivalent-byte stores only
                for kk in range(KK):
                    b = banks.tile([128, 2, IC * D], F8, tag="bank", name=f"wd{kk}")
                    nc.sync.dma_start(b[:], wd8_d.ap()[kk])
                for kk in range(KK):
                    b = banks.tile([128, 2, OC * H], F8, tag="bank", name=f"wu{kk}")
                    nc.sync.dma_start(b[:], wu8_d.ap()[kk])
                for n in range(NPC):
                    for sc in range(SC):
                        xt = xtp.tile([128, IC, SCW], F16, tag=f"xt{n}_{sc}",
                                      name=f"xt{n}_{sc}")
                        nc.sync.dma_start(xt[:], xT_d.ap()[n, sc])
                        for hc in range(IC):
                            ob = obp.tile([128, SCW], F16, tag="ob")
                            nc.vector.tensor_copy(ob[:, 0:8], xt[:, hc, 0:8])
                            nc.gpsimd.dma_start(
                                out_d.ap()[n, hc, :, sc * SCW:(sc + 1) * SCW],
                                ob[:])
                return
            SKIP_DMA = (ABL == "compute_only")

            # ---- bank loads (fp8 pairs), x chunks, tuned order ----
            wd_t, wu_t = [], []
            for kk in range(KK):
                b = banks.tile([128, 2, IC * D], F8, tag="bank", name=f"wd{kk}")
                if not SKIP_DMA:
                    nc.sync.dma_start(b[:], wd8_d.ap()[kk])
                else:
                    nc.gpsimd.memset(b[:, 0, 0:8], 0)
                wd_t.append(b)
            xt = {}
            for n in range(NPC):
                for sc in range(SC):
                    xt[(n, sc)] = xtp.tile([128, IC, SCW], F16, tag=f"xt{n}_{sc}",
                                           name=f"xt{n}_{sc}")
            if not SKIP_DMA:
                nc.sync.dma_start(xt[(0, 0)][:], xT_d.ap()[0, 0])
            for kk in range(KK):
                b = banks.tile([128, 2, OC * H], F8, tag="bank", name=f"wu{kk}")
                if not SKIP_DMA:
                    nc.sync.dma_start(b[:], wu8_d.ap()[kk])
                else:
                    nc.gpsimd.memset(b[:, 0, 0:8], 0)
                wu_t.append(b)
            if not SKIP_DMA:
                for sc in range(1, SC):
                    nc.sync.dma_start(xt[(0, sc)][:], xT_d.ap()[0, sc])
                for sc in range(SC):
                    nc.sync.dma_start(xt[(1, sc)][:], xT_d.ap()[1, sc])
            else:
                for n in range(NPC):
                    for sc in range(SC):
                        nc.gpsimd.memset(xt[(n, sc)][:, 0, 0:8], 0)

            # ---- merged biases (tiny fp32 matmuls) ----
            # mbd[:, oc*NPC+n] = merged b_down at d = oc*128+p, instance n
            # mbu[:, hc*NPC+n] = merged b_up   at h = hc*128+p, instance n
            mbd_t = work.tile([128, OC * NPC], F32, tag="mbd")
            mbu_t = work.tile([128, IC * NPC], F32, tag="mbu")
            for oc in range(OC):
                pst = pstiny.tile([128, NPC], F32, tag="pst", name="psbd")
                nc.tensor.matmul(pst[:], bd_t[:, oc * 128:(oc + 1) * 128], pkn_t[:])
                nc.vector.tensor_copy(mbd_t[:, oc * NPC:(oc + 1) * NPC], pst[:])
            for hc in range(IC):
                pst = pstiny.tile([128, NPC], F32, tag="pst", name="psbu")
                nc.tensor.matmul(pst[:], bu_t[:, hc * 128:(hc + 1) * 128], pkn_t[:])
                nc.vector.tensor_copy(mbu_t[:, hc * NPC:(hc + 1) * NPC], pst[:])

            # ---- expert-bank merges on PE (scaled-identity DoubleRow) ----
            # merged[m, f] = sum_kk sum_i p8[n,2kk+i] * bank[kk][m, i, f]
            wdm = [work.tile([128, IC, D], F8, tag=f"wdm{n}", name=f"wdm{n}")
                   for n in range(NPC)]
            wum = [work.tile([128, OC, H], F8, tag=f"wum{n}", name=f"wum{n}")
                   for n in range(NPC)]
            for n in range(NPC):
                for c in range(IC * D // MCW):      # wd chunks: (2 ic) x D
                    pm = psm.tile([128, 2, MCW // 2], F32, tag="psm", name="psmd")
                    for kk in range(KK):
                        nc.tensor.matmul(
                            pm[:], pid_t[:, n, kk, :, :],
                            wd_t[kk][:, :, c * MCW:(c + 1) * MCW],
                            start=(kk == 0), stop=(kk == KK - 1), perf_mode=DR)
                    nc.vector.tensor_copy(wdm[n][:, 2 * c:2 * c + 2, :], pm[:])
                for c in range(OC * H // MCW):      # wu chunks: (oc, h-half)
                    pm = psm.tile([128, MCW], F32, tag="psm", name="psmu")
                    for kk in range(KK):
                        nc.tensor.matmul(
                            pm[:], pid_t[:, n, kk, :, :],
                            wu_t[kk][:, :, c * MCW:(c + 1) * MCW],
                            start=(kk == 0), stop=(kk == KK - 1), perf_mode=DR)
                    nc.vector.tensor_copy(
                        wum[n][:, c // 2, (c % 2) * MCW:(c % 2) * MCW + MCW], pm[:])

            # ---- per instance: mm1 -> relu1 (fp8), mm2 (DoubleRow) -> epilogue
            for n in range(NPC):
                relu1 = work.tile([128, OC, S], F8, tag=f"relu{n}", name=f"relu{n}")
                tile_i = 0
                for sc in range(SC):
                    for oc in range(OC):
                        p1 = ps1p.tile([128, SCW], F32, tag="ps1")
                        for ic in range(IC):
                            nc.tensor.matmul(
                                p1[:],
                                wdm[n][:, ic, oc * 128:(oc + 1) * 128],
                                xt[(n, sc)][:, ic, :],
                                start=(ic == 0), stop=(ic == IC - 1))
                        nc.scalar.activation(
                            relu1[:, oc, sc * SCW:(sc + 1) * SCW], p1[:],
                            mybir.ActivationFunctionType.Relu,
                            bias=mbd_t[:, oc * NPC + n:oc * NPC + n + 1], scale=1.0)
                    for hc in range(IC):
                        p2 = ps2p.tile([128, SCW], F32, tag="ps2")
                        nc.tensor.matmul(
                            p2[:],
                            wum[n][:, :, hc * 128:(hc + 1) * 128],
                            relu1[:, :, sc * SCW:(sc + 1) * SCW],
                            start=True, stop=True, perf_mode=DR)
                        ob = obp.tile([128, SCW], F16, tag="ob")
                        mbu_ap = mbu_t[:, hc * NPC + n:hc * NPC + n + 1]
                        if (tile_i % 100) < int(OPTS["dve_frac"] * 100):
                            # fused: out = (psum + bu) + xT  on DVE
                            nc.vector.scalar_tensor_tensor(
                                ob[:], p2[:], mbu_ap, xt[(n, sc)][:, hc, :],
                                mybir.AluOpType.add, mybir.AluOpType.add)
                        else:
                            tmp = obp.tile([128, SCW], F16, tag="obtmp")
                            nc.scalar.activation(
                                tmp[:], p2[:],
                                mybir.ActivationFunctionType.Copy,
                                bias=mbu_ap, scale=1.0)
                            nc.gpsimd.tensor_tensor(
                                ob[:], tmp[:], xt[(n, sc)][:, hc, :],
                                mybir.AluOpType.add)
                        tile_i += 1
                        if not SKIP_DMA:
                            nc.gpsimd.dma_start(
                                out_d.ap()[n, hc, :, sc * SCW:(sc + 1) * SCW],
                                ob[:])


def build(loop_t=None):
    """Build and compile the per-core NEFF. Cached per loop_t."""
    key = (loop_t, OPTS["ablate"], OPTS["dve_frac"])
    if key in _CACHE:
        return _CACHE[key]
    nc = bacc.Bacc("TRN2", target_bir_lowering=False, debug=False,
                   num_devices=NCORES)
    tens = (
        nc.dram_tensor("xT", [NPC, SC, 128, IC, SCW], F16, kind="ExternalInput"),
        nc.dram_tensor("wd8", [KK, 128, 2, IC * D], F8, kind="ExternalInput"),
        nc.dram_tensor("wu8", [KK, 128, 2, OC * H], F8, kind="ExternalInput"),
        nc.dram_tensor("pid", [128, NPC, KK, 2, 128], F8, kind="ExternalInput"),
        nc.dram_tensor("bd", [K, D], F32, kind="ExternalInput"),
        nc.dram_tensor("bu", [K, H], F32, kind="ExternalInput"),
        nc.dram_tensor("pkn", [K, NPC], F32, kind="ExternalInput"),
        nc.dram_tensor("out", [NPC, IC, 128, S], F16, kind="ExternalOutput"),
    )
    with tile.TileContext(nc) as tc:
        _emit(nc, tc, tens, loop_t=loop_t)
    nc.compile()
    _CACHE[key] = nc
    return nc


def make_in_maps(hidden_states, prob, w_down, b_down, w_up, b_up):
    """Shard + lay out the full inputs for the 8 cores."""
    f = np.float32
    hs = np.asarray(hidden_states, dtype=f)
    prob = np.asarray(prob, dtype=f)
    # banks, interleaved expert pairs for DoubleRow:
    # wd8[kk, p, i, ic*D+d] = w_down[2kk+i, d, ic*128+p]
    wd8 = (np.asarray(w_down, f).transpose(2, 0, 1)      # (H, K, D)
           .reshape(IC, 128, KK, 2, D).transpose(2, 1, 3, 0, 4)
           .reshape(KK, 128, 2, IC * D)).astype(np8)
    # wu8[kk, p, i, oc*H+h] = w_up[2kk+i, h, oc*128+p]
    wu8 = (np.asarray(w_up, f).transpose(2, 0, 1)        # (D, K, H)
           .reshape(OC, 128, KK, 2, H).transpose(2, 1, 3, 0, 4)
           .reshape(KK, 128, 2, OC * H)).astype(np8)
    bd = np.ascontiguousarray(np.asarray(b_down, f))
    bu = np.ascontiguousarray(np.asarray(b_up, f))
    eye = np.eye(128, dtype=f)
    in_maps = []
    for c in range(NCORES):
        shard = hs[c * NPC:(c + 1) * NPC]                 # (NPC, S, H)
        p_shard = prob[c * NPC:(c + 1) * NPC]             # (NPC, K)
        # xT[n, sc, p, ic, j] = x[n, sc*SCW+j, ic*128+p]
        xT = (shard.reshape(NPC, SC, SCW, IC, 128)
              .transpose(0, 1, 4, 3, 2)).astype(np16)
        # pid[p, n, kk, i, m] = fp8(prob[n, 2kk+i]) * eye[p, m]
        p8 = p_shard.astype(np8).astype(f)                # quantized probs
        pid = (p8.reshape(1, NPC, KK, 2, 1) *
               eye.reshape(128, 1, 1, 1, 128)).astype(np8)
        in_maps.append({
            "xT": np.ascontiguousarray(xT),
            "wd8": wd8,
            "wu8": wu8,
            "pid": pid,
            "bd": bd,
            "bu": bu,
            "pkn": np.ascontiguousarray(p_shard.T),
        })
    return in_maps


def kernel(hidden_states, prob, w_down, b_down, w_up, b_up):
    nc = build()
    in_maps = make_in_maps(hidden_states, prob, w_down, b_down, w_up, b_up)
    res = run_bass_kernel_spmd(nc, in_maps, list(range(NCORES)))
    # out'[n, hc, p, s] -> out[n, s, hc*128+p]
    out = np.stack([res.results[c]["out"] for c in range(NCORES)], axis=0)
    out = out.reshape(N, IC * 128, S).transpose(0, 2, 1)
    return np.ascontiguousarray(out.astype(np.float32))
